# revision 13
# baseline (speedup 1.0000x reference)
"""Trainium2 Bass kernel for IR-Net style binarized 3x3 conv + BN + Hardtanh.

Reference computation:
  bw = sign(standardize(weight)) * sw   (sw = per-cout power-of-2 scale)
  ba = sign(x)
  y  = clip(conv3x3(ba, bw) * bn_scale + bn_bias, -1, 1)

Both matmul operands are exactly +-1, which is exactly representable in
fp8e4m3, so the conv runs as fp8 DoubleRow matmuls on the TensorEngine
with zero numerical error (fp32 PSUM accumulation of integers <= 2304).
Weight standardization/sign, sw, and BN folding are host-side prep
(0.6 MB of data); sw and bn scale fold into a single per-channel scale
applied in the epilogue (on VectorE, so ScalarE is free for binarize).

Distribution: pure data parallel, 32 images -> 4 per NeuronCore, full
weights replicated, no collectives.

Layout: per-image zero-padded 58x58 activation planes in SBUF, fp8, with
the two cin-128-chunks stacked as the DoubleRow k-subtile dim.  Each of
the 9 conv taps is then a contiguous shifted window of the flattened
padded plane, so the conv becomes 9 accumulated DoubleRow matmuls
([128,2,128] @ [128,2,464], K=256) per 8-row output tile.
"""

import numpy as np

import concourse.bass as bass
import concourse.bacc as bacc
import concourse.mybir as mybir
import concourse.tile as tile
from concourse.bass_utils import run_bass_kernel_spmd

B, CIN, COUT, H, W = 32, 256, 256, 56, 56
NCORES = 8
BPC = B // NCORES            # images per core
HP, WP = H + 2, W + 2        # zero-padded plane
IMG = HP * WP                # 3364
GUARD = 64                   # front zero guard (shifted windows stay in bounds)
XT = 3504                    # GUARD + IMG + tail guard(76); %16==0 for DoubleRow
RB = 8                       # output rows per tile
NBLK = H // RB               # 7
NT = RB * WP                 # 464 matmul free dim (incl. 2 garbage cols/row)
NCI = CIN // 128             # 2 cin chunks = DoubleRow k-subtiles
NCO = COUT // 128            # 2 cout chunks
KTAPS = 9
BN_EPS = 1e-5

F32 = mybir.dt.float32
FP8 = mybir.dt.float8e4

_CACHE: dict = {}


def _build_nc() -> bass.Bass:
    nc = bacc.Bacc("TRN2", target_bir_lowering=False, debug=False, num_devices=NCORES)
    xin = nc.declare_dram_parameter("xin", [BPC, CIN, H * W], F32, isOutput=False)
    wts = nc.declare_dram_parameter(
        "wts", [128, KTAPS * NCO * NCI * 128], FP8, isOutput=False
    )
    sb = nc.declare_dram_parameter("sb", [128, 2 * NCO], F32, isOutput=False)
    yout = nc.declare_dram_parameter("yout", [BPC, COUT, H, W], F32, isOutput=True)

    RCH = 14                 # input rows per binarize chunk (image 0 only)
    NCH = H // RCH           # 4 chunks per (img0, cin-chunk)

    with tile.TileContext(nc) as tc:
        with (
            tc.tile_pool(name="const", bufs=1) as cpool,
            tc.tile_pool(name="stage_s", bufs=8) as spool_s,
            tc.tile_pool(name="stage_l", bufs=6) as spool_l,
            tc.tile_pool(name="psum", bufs=6, space=bass.MemorySpace.PSUM) as ppool,
            tc.tile_pool(name="out", bufs=4) as opool,
        ):
            # weights: [p, (k, co, j, m)]
            w_sb = cpool.tile([128, KTAPS * NCO * NCI * 128], FP8, tag="w")
            sb_sb = cpool.tile([128, 2 * NCO], F32, tag="sb")
            nc.sync.dma_start(w_sb[:], wts[:])
            nc.sync.dma_start(sb_sb[:], sb[:])
            w4 = w_sb.rearrange("p (k co j m) -> p k co j m", k=KTAPS, co=NCO, j=NCI)

            # Padded binarized activation planes, one tile per image.  The
            # two cin-128-chunks (DoubleRow k-subtiles) are interleaved
            # byte-wise as the innermost dim so every matmul rhs window is a
            # tight flat byte range (keeps RAW dep tracking per row-band).
            xp = {}
            for img in range(BPC):
                t = cpool.tile([128, XT, NCI], FP8, tag=f"xp{img}")
                xp[img] = t
                for j in range(NCI):
                    # zero only the borders: top guard+row0, bottom row57+tail
                    # guard, and the two side columns of rows 1..56.
                    nc.gpsimd.memset(t[:, 0 : GUARD + WP, j], 0.0)
                    nc.gpsimd.memset(t[:, GUARD + (HP - 1) * WP : XT, j], 0.0)
                    side = t[:, GUARD + WP : GUARD + WP + H * WP, j].rearrange(
                        "p (h w) -> p h w", w=WP
                    )
                    nc.gpsimd.memset(side[:, :, 0:1], 0.0)
                    nc.gpsimd.memset(side[:, :, WP - 1 : WP], 0.0)

            def plane_view(img, j):
                return xp[img][:, GUARD : GUARD + IMG, j].rearrange(
                    "p (h w) -> p h w", w=WP
                )

            # Image 0 gated fine-grained (the first matmuls wait on it):
            # row chunks, j-interleaved so the first chunks issue first.
            for c in range(NCH):
                for j in range(NCI):
                    r0 = c * RCH
                    st = spool_s.tile([128, RCH * W], F32, tag="stage_s")
                    nc.sync.dma_start(
                        st[:],
                        xin[0, j * 128 : (j + 1) * 128, r0 * W : (r0 + RCH) * W],
                    )
                    interior = plane_view(0, j)[:, 1 + r0 : 1 + r0 + RCH, 1 : W + 1]
                    nc.scalar.sign(interior, st.rearrange("p (h w) -> p h w", w=W))

            # Images 1..3: whole-plane loads (fewer DMA descriptors on Sync).
            for img in range(1, BPC):
                for j in range(NCI):
                    st = spool_l.tile([128, H * W], F32, tag="stage_l")
                    nc.sync.dma_start(st[:], xin[img, j * 128 : (j + 1) * 128, :])
                    interior = plane_view(img, j)[:, 1 : H + 1, 1 : W + 1]
                    nc.scalar.sign(interior, st.rearrange("p (h w) -> p h w", w=W))

            for img in range(BPC):
                for co in range(NCO):
                    s_ap = sb_sb[:, co : co + 1]
                    b_ap = sb_sb[:, NCO + co : NCO + co + 1]
                    for blk in range(NBLK):
                        y0p = 1 + blk * RB  # first padded row of this block
                        ps = ppool.tile([128, NT], F32, tag="ps")
                        for k in range(KTAPS):
                            ky, kx = divmod(k, 3)
                            s0 = GUARD + (y0p + ky - 1) * WP + (kx - 1)
                            rhs = xp[img][:, s0 : s0 + NT, :].rearrange(
                                "p x j -> p j x"
                            )
                            nc.tensor.matmul(
                                ps[:],
                                w4[:, k, co],
                                rhs,
                                start=(k == 0),
                                stop=(k == KTAPS - 1),
                                perf_mode=mybir.MatmulPerfMode.DoubleRow,
                            )
                        ot = opool.tile([128, NT], F32, tag="ot")
                        nc.vector.tensor_scalar(
                            ot[:],
                            ps[:],
                            s_ap,
                            b_ap,
                            op0=mybir.AluOpType.mult,
                            op1=mybir.AluOpType.add,
                        )
                        # clip + compact away the 2 garbage cols per row, so
                        # both sides of the output DMA are fully contiguous
                        oc = opool.tile([128, RB * W], F32, tag="oc")
                        nc.vector.tensor_scalar(
                            oc[:],
                            ot.rearrange("p (r c) -> p r c", c=WP)[:, :, 1 : W + 1],
                            -1.0,
                            1.0,
                            op0=mybir.AluOpType.max,
                            op1=mybir.AluOpType.min,
                        )
                        nc.sync.dma_start(
                            yout[img, co * 128 : (co + 1) * 128, y0p - 1 : y0p - 1 + RB, :],
                            oc[:],
                        )
    nc.finalize()
    return nc


def get_nc() -> bass.Bass:
    if "nc" not in _CACHE:
        _CACHE["nc"] = _build_nc()
    return _CACHE["nc"]


def _host_prep(weight, gamma, beta, running_mean, running_var):
    """Binarize standardized weights, fold sw + BN into scale/bias."""
    wf = weight.reshape(COUT, -1).astype(np.float64)
    n = wf.shape[1]
    mean = wf.mean(axis=1, keepdims=True)
    d = wf - mean
    sgn = np.where(d >= 0, 1.0, -1.0)
    std = np.sqrt((d * d).sum(axis=1, keepdims=True) / (n - 1))
    bw = d / std
    sw = np.exp2(np.round(np.log2(np.abs(bw).mean(axis=1))))  # [COUT]
    inv = gamma.astype(np.float64) / np.sqrt(running_var.astype(np.float64) + BN_EPS)
    scale = (sw * inv).astype(np.float32)
    bias = (beta.astype(np.float64) - running_mean.astype(np.float64) * inv).astype(
        np.float32
    )

    # wts[p, (k, co, j, m)] = sgn[co*128+m, (j*128+p)*9 + k]
    fp8np = mybir.dt.np(FP8)
    w6 = sgn.reshape(NCO, 128, NCI, 128, KTAPS)  # [co, m, j, p, k]
    wts = (
        np.ascontiguousarray(np.transpose(w6, (3, 4, 0, 2, 1)))  # p k co j m
        .reshape(128, KTAPS * NCO * NCI * 128)
        .astype(fp8np)
    )
    # sb[m, co] = scale chunk, sb[m, NCO+co] = bias chunk
    sbarr = np.concatenate(
        [scale.reshape(NCO, 128).T, bias.reshape(NCO, 128).T], axis=1
    ).astype(np.float32)
    sbarr = np.ascontiguousarray(sbarr)
    return wts, sbarr


def run(x, weight, gamma, beta, running_mean, running_var, trace=False, **tkw):
    x = np.asarray(x, dtype=np.float32)
    wts, sbarr = _host_prep(
        np.asarray(weight, dtype=np.float32),
        np.asarray(gamma, dtype=np.float32),
        np.asarray(beta, dtype=np.float32),
        np.asarray(running_mean, dtype=np.float32),
        np.asarray(running_var, dtype=np.float32),
    )
    in_maps = [
        {
            "xin": np.ascontiguousarray(
                x[c * BPC : (c + 1) * BPC].reshape(BPC, CIN, H * W)
            ),
            "wts": wts,
            "sb": sbarr,
        }
        for c in range(NCORES)
    ]
    nc = get_nc()
    res = run_bass_kernel_spmd(nc, in_maps, list(range(NCORES)), trace=trace, **tkw)
    y = np.concatenate([r["yout"] for r in res.results], axis=0)
    return y.astype(np.float32, copy=False), res


def kernel(x, weight, gamma, beta, running_mean, running_var):
    y, _ = run(x, weight, gamma, beta, running_mean, running_var)
    return y


# revision 14
# speedup vs baseline: 1.0640x; 1.0640x over previous
"""Trainium2 Bass kernel for IR-Net style binarized 3x3 conv + BN + Hardtanh.

Reference computation:
  bw = sign(standardize(weight)) * sw   (sw = per-cout power-of-2 scale)
  ba = sign(x)
  y  = clip(conv3x3(ba, bw) * bn_scale + bn_bias, -1, 1)

Both matmul operands are exactly +-1, which is exactly representable in
fp8e4m3, so the conv runs as fp8 DoubleRow matmuls on the TensorEngine
with zero numerical error (fp32 PSUM accumulation of integers <= 2304).
Weight standardization/sign, sw, and BN folding are host-side prep
(0.6 MB of data); sw and bn scale fold into a single per-channel scale
applied in the epilogue (on VectorE, so ScalarE is free for binarize).

Distribution: pure data parallel, 32 images -> 4 per NeuronCore, full
weights replicated, no collectives.

Layout: per-image zero-padded 58x58 activation planes in SBUF, fp8, with
the two cin-128-chunks stacked as the DoubleRow k-subtile dim.  Each of
the 9 conv taps is then a contiguous shifted window of the flattened
padded plane, so the conv becomes 9 accumulated DoubleRow matmuls
([128,2,128] @ [128,2,464], K=256) per 8-row output tile.
"""

import numpy as np

import concourse.bass as bass
import concourse.bacc as bacc
import concourse.mybir as mybir
import concourse.tile as tile
from concourse.bass_utils import run_bass_kernel_spmd

B, CIN, COUT, H, W = 32, 256, 256, 56, 56
NCORES = 8
BPC = B // NCORES            # images per core
HP, WP = H + 2, W + 2        # zero-padded plane
IMG = HP * WP                # 3364
GUARD = 64                   # front zero guard (shifted windows stay in bounds)
XT = 3504                    # GUARD + IMG + tail guard(76); %16==0 for DoubleRow
RB = 8                       # output rows per tile
NBLK = H // RB               # 7
NT = RB * WP                 # 464 matmul free dim (incl. 2 garbage cols/row)
NCI = CIN // 128             # 2 cin chunks = DoubleRow k-subtiles
NCO = COUT // 128            # 2 cout chunks
KTAPS = 9
BN_EPS = 1e-5

F32 = mybir.dt.float32
FP8 = mybir.dt.float8e4
BF16 = mybir.dt.bfloat16

_CACHE: dict = {}


def _build_nc() -> bass.Bass:
    nc = bacc.Bacc("TRN2", target_bir_lowering=False, debug=False, num_devices=NCORES)
    xin = nc.declare_dram_parameter("xin", [BPC, CIN, H * W], BF16, isOutput=False)
    wts = nc.declare_dram_parameter(
        "wts", [128, KTAPS * NCO * NCI * 128], FP8, isOutput=False
    )
    sb = nc.declare_dram_parameter("sb", [128, 2 * NCO], F32, isOutput=False)
    yout = nc.declare_dram_parameter("yout", [BPC, COUT, H, W], F32, isOutput=True)

    RCH = 14                 # input rows per binarize chunk (image 0 only)
    NCH = H // RCH           # 4 chunks per (img0, cin-chunk)

    with tile.TileContext(nc) as tc:
        with (
            tc.tile_pool(name="const", bufs=1) as cpool,
            tc.tile_pool(name="stage_s", bufs=8) as spool_s,
            tc.tile_pool(name="stage_l", bufs=6) as spool_l,
            tc.tile_pool(name="psum", bufs=8, space=bass.MemorySpace.PSUM) as ppool,
            tc.tile_pool(name="ot", bufs=8) as otpool,
            tc.tile_pool(name="oc", bufs=12) as ocpool,
        ):
            # weights: [p, (k, co, j, m)]
            w_sb = cpool.tile([128, KTAPS * NCO * NCI * 128], FP8, tag="w")
            sb_sb = cpool.tile([128, 2 * NCO], F32, tag="sb")
            nc.sync.dma_start(w_sb[:], wts[:])
            nc.sync.dma_start(sb_sb[:], sb[:])
            w4 = w_sb.rearrange("p (k co j m) -> p k co j m", k=KTAPS, co=NCO, j=NCI)

            # Padded binarized activation planes, one tile per image.  The
            # two cin-128-chunks (DoubleRow k-subtiles) are interleaved
            # byte-wise as the innermost dim so every matmul rhs window is a
            # tight flat byte range (keeps RAW dep tracking per row-band).
            xp = {}
            for img in range(BPC):
                t = cpool.tile([128, XT, NCI], FP8, tag=f"xp{img}")
                xp[img] = t
                for j in range(NCI):
                    # zero only the borders: top guard+row0, bottom row57+tail
                    # guard, and the two side columns of rows 1..56.
                    nc.gpsimd.memset(t[:, 0 : GUARD + WP, j], 0.0)
                    nc.gpsimd.memset(t[:, GUARD + (HP - 1) * WP : XT, j], 0.0)
                    side = t[:, GUARD + WP : GUARD + WP + H * WP, j].rearrange(
                        "p (h w) -> p h w", w=WP
                    )
                    nc.gpsimd.memset(side[:, :, 0:1], 0.0)
                    nc.gpsimd.memset(side[:, :, WP - 1 : WP], 0.0)

            def plane_view(img, j):
                return xp[img][:, GUARD : GUARD + IMG, j].rearrange(
                    "p (h w) -> p h w", w=WP
                )

            # Image 0 gated fine-grained (the first matmuls wait on it):
            # row chunks, j-interleaved so the first chunks issue first.
            for c in range(NCH):
                for j in range(NCI):
                    r0 = c * RCH
                    st = spool_s.tile([128, RCH * W], BF16, tag="stage_s")
                    nc.sync.dma_start(
                        st[:],
                        xin[0, j * 128 : (j + 1) * 128, r0 * W : (r0 + RCH) * W],
                    )
                    interior = plane_view(0, j)[:, 1 + r0 : 1 + r0 + RCH, 1 : W + 1]
                    nc.scalar.sign(interior, st.rearrange("p (h w) -> p h w", w=W))

            # Images 1..3: whole-plane loads (fewer DMA descriptors on Sync).
            for img in range(1, BPC):
                for j in range(NCI):
                    st = spool_l.tile([128, H * W], BF16, tag="stage_l")
                    nc.sync.dma_start(st[:], xin[img, j * 128 : (j + 1) * 128, :])
                    interior = plane_view(img, j)[:, 1 : H + 1, 1 : W + 1]
                    nc.scalar.sign(interior, st.rearrange("p (h w) -> p h w", w=W))

            for img in range(BPC):
                for co in range(NCO):
                    s_ap = sb_sb[:, co : co + 1]
                    b_ap = sb_sb[:, NCO + co : NCO + co + 1]
                    for blk in range(NBLK):
                        y0p = 1 + blk * RB  # first padded row of this block
                        ps = ppool.tile([128, NT], F32, tag="ps")
                        for k in range(KTAPS):
                            ky, kx = divmod(k, 3)
                            s0 = GUARD + (y0p + ky - 1) * WP + (kx - 1)
                            rhs = xp[img][:, s0 : s0 + NT, :].rearrange(
                                "p x j -> p j x"
                            )
                            nc.tensor.matmul(
                                ps[:],
                                w4[:, k, co],
                                rhs,
                                start=(k == 0),
                                stop=(k == KTAPS - 1),
                                perf_mode=mybir.MatmulPerfMode.DoubleRow,
                            )
                        ot = otpool.tile([128, NT], F32, tag="ot")
                        nc.vector.tensor_scalar(
                            ot[:],
                            ps[:],
                            s_ap,
                            b_ap,
                            op0=mybir.AluOpType.mult,
                            op1=mybir.AluOpType.add,
                        )
                        # clip + compact away the 2 garbage cols per row, so
                        # both sides of the output DMA are fully contiguous
                        oc = ocpool.tile([128, RB * W], F32, tag="oc")
                        nc.vector.tensor_scalar(
                            oc[:],
                            ot.rearrange("p (r c) -> p r c", c=WP)[:, :, 1 : W + 1],
                            -1.0,
                            1.0,
                            op0=mybir.AluOpType.max,
                            op1=mybir.AluOpType.min,
                        )
                        nc.sync.dma_start(
                            yout[img, co * 128 : (co + 1) * 128, y0p - 1 : y0p - 1 + RB, :],
                            oc[:],
                        )
    nc.finalize()
    return nc


def get_nc() -> bass.Bass:
    if "nc" not in _CACHE:
        _CACHE["nc"] = _build_nc()
    return _CACHE["nc"]


def _host_prep(weight, gamma, beta, running_mean, running_var):
    """Binarize standardized weights, fold sw + BN into scale/bias."""
    wf = weight.reshape(COUT, -1).astype(np.float64)
    n = wf.shape[1]
    mean = wf.mean(axis=1, keepdims=True)
    d = wf - mean
    sgn = np.where(d >= 0, 1.0, -1.0)
    std = np.sqrt((d * d).sum(axis=1, keepdims=True) / (n - 1))
    bw = d / std
    sw = np.exp2(np.round(np.log2(np.abs(bw).mean(axis=1))))  # [COUT]
    inv = gamma.astype(np.float64) / np.sqrt(running_var.astype(np.float64) + BN_EPS)
    scale = (sw * inv).astype(np.float32)
    bias = (beta.astype(np.float64) - running_mean.astype(np.float64) * inv).astype(
        np.float32
    )

    # wts[p, (k, co, j, m)] = sgn[co*128+m, (j*128+p)*9 + k]
    fp8np = mybir.dt.np(FP8)
    w6 = sgn.reshape(NCO, 128, NCI, 128, KTAPS)  # [co, m, j, p, k]
    wts = (
        np.ascontiguousarray(np.transpose(w6, (3, 4, 0, 2, 1)))  # p k co j m
        .reshape(128, KTAPS * NCO * NCI * 128)
        .astype(fp8np)
    )
    # sb[m, co] = scale chunk, sb[m, NCO+co] = bias chunk
    sbarr = np.concatenate(
        [scale.reshape(NCO, 128).T, bias.reshape(NCO, 128).T], axis=1
    ).astype(np.float32)
    sbarr = np.ascontiguousarray(sbarr)
    return wts, sbarr


def run(x, weight, gamma, beta, running_mean, running_var, trace=False, **tkw):
    x = np.asarray(x, dtype=np.float32)
    wts, sbarr = _host_prep(
        np.asarray(weight, dtype=np.float32),
        np.asarray(gamma, dtype=np.float32),
        np.asarray(beta, dtype=np.float32),
        np.asarray(running_mean, dtype=np.float32),
        np.asarray(running_var, dtype=np.float32),
    )
    import ml_dtypes

    # bf16 truncation of x preserves every sign bit (min |x| >> bf16 denormal
    # range), and sign() is all the kernel reads from x — halves input DMA.
    xb = np.ascontiguousarray(
        x.reshape(B, CIN, H * W).view(np.uint16)[..., 1::2]
    ).view(ml_dtypes.bfloat16)
    in_maps = [
        {
            "xin": xb[c * BPC : (c + 1) * BPC],
            "wts": wts,
            "sb": sbarr,
        }
        for c in range(NCORES)
    ]
    nc = get_nc()
    res = run_bass_kernel_spmd(nc, in_maps, list(range(NCORES)), trace=trace, **tkw)
    y = np.concatenate([r["yout"] for r in res.results], axis=0)
    return y.astype(np.float32, copy=False), res


def kernel(x, weight, gamma, beta, running_mean, running_var):
    y, _ = run(x, weight, gamma, beta, running_mean, running_var)
    return y
